# revision 1
# baseline (speedup 1.0000x reference)
"""Two-layer GCN (PyG GCNConv semantics) on 8 Trainium2 NeuronCores.

Strategy (1D graph partitioning, destination-sharded):
  * Nodes are sorted by in-degree (descending), padded to a multiple of
    128*8, and chunked into groups of 128.  Group g is owned by core g%8.
    Node identity on device = "table row" t = k*(J*128) + p*J + j for core
    k, partition slot p, local group j.
  * All per-edge index work happens on the host: each destination node
    gets Dhat_j padded edge slots; slot (p, d) of a group holds the edge
    weight w_e and the table row of the source node.  Padding slots have
    w=0 and point at row 0.
  * On device, per layer:  gather source rows with indirect DMA from a
    DRAM table (f32 rows, 256B descriptors), multiply by
    w~ = w * dinv[dst] (broadcast over features), and reduce over the
    edge-slot axis with a strided DVE reduction.  Aggregation runs before
    the 64x64 weight matmul ((A x) W == A (x W)), so only J tiles need the
    transpose + matmul.  dinv[src] is folded into the gather table
    (x' = dinv * x), recomputed per layer; dinv[dst] is folded into w~.
  * The table for layer l+1 is built with one 8-core AllGather of the
    dinv-scaled layer output.
"""

import math
import sys
from contextlib import ExitStack

import numpy as np

if "/opt/trn_rl_repo" not in sys.path:
    sys.path.insert(0, "/opt/trn_rl_repo")

P = 128  # SBUF partitions
C = 8    # NeuronCores
F = 64   # feature width (in = hidden = out = 64)
GATHER_SLOT_BUDGET = 64  # max padded edge slots per gather batch (per partition)
WAVE = 8                 # groups per transform wave (8*64 = 512 = one PSUM bank)


# ---------------------------------------------------------------------------
# Host-side graph preprocessing (integer index work + permutations only)
# ---------------------------------------------------------------------------

def _plan(n_nodes, edge_index, edge_feats):
    N = int(n_nodes)
    G0 = math.ceil(N / P)
    G_total = math.ceil(G0 / C) * C
    J = G_total // C
    N_pad = G_total * P

    row = np.asarray(edge_index[0], dtype=np.int64)
    col = np.asarray(edge_index[1], dtype=np.int64)
    w = np.asarray(edge_feats, dtype=np.float32)

    # Self-loops are NOT materialized as edge slots: the self contribution
    # dinv[v]^2 * x[v] is added on-device from the SBUF-resident slice.
    r_all = row
    c_all = col
    w_all = w

    degc = np.bincount(c_all, minlength=N_pad)  # real in-degree (may be 0)
    order = np.argsort(-degc, kind="stable")    # descending degree
    s_of = np.empty(N_pad, np.int64)
    s_of[order] = np.arange(N_pad)
    g_of = s_of // P
    p_of = s_of % P
    k_of = g_of % C
    j_of = g_of // C
    t_of = k_of * (P * J) + p_of * J + j_of     # table row per node

    # per-group max degree; descending order => stripe max is the first one
    Dg = degc[order[np.arange(G_total) * P]]
    Dhat = Dg[0::C].astype(np.int64)  # [J], may be 0 for the tail
    off = np.concatenate([[0], np.cumsum(Dhat)]).astype(np.int64)
    SD = int(off[-1])

    # edge slot assignment: sort edges by destination table row
    tdst = t_of[c_all]
    oE = np.argsort(tdst, kind="stable")
    td = tdst[oE]
    dslot = np.arange(len(td), dtype=np.int64) - np.searchsorted(td, td, side="left")
    kk = td // (P * J)
    rem = td - kk * (P * J)
    pp = rem // J
    jj = rem - pp * J
    assert np.all(dslot < Dhat[jj]), "edge slot exceeded padded degree"

    w_pad = np.zeros((C, P, SD), np.float32)
    idx = np.zeros((C, P, SD), np.int32)
    colpos = off[jj] + dslot
    w_pad[kk, pp, colpos] = w_all[oE]
    idx[kk, pp, colpos] = t_of[r_all[oE]].astype(np.int32)

    # gather batches: consecutive groups packed to <= GATHER_SLOT_BUDGET slots
    batches = []  # (j0, j1, off0, off1)
    j0 = 0
    while j0 < J:
        j1 = j0 + 1
        while j1 < J and off[j1 + 1] - off[j0] <= GATHER_SLOT_BUDGET:
            j1 += 1
        if off[j1] > off[j0]:  # skip fully-empty tails
            batches.append((j0, j1, int(off[j0]), int(off[j1])))
        j0 = j1

    return dict(N=N, N_pad=N_pad, J=J, SD=SD, Dhat=Dhat, off=off, t_of=t_of,
                w_pad=w_pad, idx=idx, batches=batches)


def _shard_x(node_feats, plan):
    N, N_pad, J = plan["N"], plan["N_pad"], plan["J"]
    x_perm = np.zeros((N_pad, F), np.float32)
    x_perm[plan["t_of"][:N]] = np.asarray(node_feats, dtype=np.float32)
    # table row t = k*(P*J) + p*J + j  ->  [C, P, J*F]
    return x_perm.reshape(C, P, J, F).reshape(C, P, J * F)


# ---------------------------------------------------------------------------
# Device program
# ---------------------------------------------------------------------------

def _build(plan):
    from concourse import bacc, bass, mybir
    import concourse.tile as tile
    from concourse.masks import make_identity

    f32 = mybir.dt.float32
    i32 = mybir.dt.int32
    J, SD = plan["J"], plan["SD"]
    Dhat, off, batches = plan["Dhat"], plan["off"], plan["batches"]
    JP = J * P
    maxS = max(o1 - o0 for (_, _, o0, o1) in batches)

    nc = bacc.Bacc(None, target_bir_lowering=False, num_devices=C)

    x_in = nc.dram_tensor("x_slice", [P, J * F], f32, kind="ExternalInput")
    w_in = nc.dram_tensor("w_pad", [P, SD], f32, kind="ExternalInput")
    idx_in = nc.dram_tensor("idx", [P, SD], i32, kind="ExternalInput")
    W1_in = nc.dram_tensor("W1", [F, F], f32, kind="ExternalInput")
    W2_in = nc.dram_tensor("W2", [F, F], f32, kind="ExternalInput")
    b1_in = nc.dram_tensor("b1", [P, F], f32, kind="ExternalInput")
    b2_in = nc.dram_tensor("b2", [P, F], f32, kind="ExternalInput")
    out_t = nc.dram_tensor("out", [P, J * F], f32, kind="ExternalOutput")

    ag1 = nc.dram_tensor("ag_in1", [JP, F], f32)
    ag2 = nc.dram_tensor("ag_in2", [JP, F], f32)
    table1 = nc.dram_tensor("table1", [C * JP, F], f32)
    table2 = nc.dram_tensor("table2", [C * JP, F], f32)

    groups = [list(range(C))]

    with ExitStack() as ctx:
        tc = ctx.enter_context(tile.TileContext(nc))
        big = ctx.enter_context(tc.tile_pool(name="big", bufs=1))
        gp = ctx.enter_context(tc.tile_pool(name="gp", bufs=4))
        aT = ctx.enter_context(tc.tile_pool(name="aT", bufs=1))
        ep = ctx.enter_context(tc.tile_pool(name="ep", bufs=2))
        pT = ctx.enter_context(tc.tile_pool(name="pT", bufs=2, space="PSUM"))
        pZ = ctx.enter_context(tc.tile_pool(name="pZ", bufs=2, space="PSUM"))

        xs = big.tile([P, J * F], f32)
        wb = big.tile([P, SD], f32)
        wt = big.tile([P, SD], f32)
        idxs = big.tile([P, SD], i32)
        deg = big.tile([P, J], f32)
        rec = big.tile([P, J], f32)
        dinv = big.tile([P, J], f32)
        b1t = big.tile([P, F], f32)
        b2t = big.tile([P, F], f32)
        W1t = big.tile([F, F], f32)
        W2t = big.tile([F, F], f32)
        ident = big.tile([P, P], f32)
        agg = big.tile([P, J * F], f32)
        zb = big.tile([P, J * F], f32)

        # ---- loads ----
        nc.sync.dma_start(out=xs[:], in_=x_in[:, :])
        nc.sync.dma_start(out=wb[:], in_=w_in[:, :])
        nc.sync.dma_start(out=idxs[:], in_=idx_in[:, :])
        nc.sync.dma_start(out=W1t[:], in_=W1_in[:, :])
        nc.sync.dma_start(out=W2t[:], in_=W2_in[:, :])
        nc.sync.dma_start(out=b1t[:], in_=b1_in[:, :])
        nc.sync.dma_start(out=b2t[:], in_=b2_in[:, :])
        make_identity(nc, ident[:])

        # ---- degrees / dinv / w~ ----
        # deg = sum of in-edge weights + 1 (the self-loop, handled separately)
        nc.vector.memset(deg[:], 0.0)
        for j in range(J):
            if off[j + 1] > off[j]:
                nc.vector.reduce_sum(
                    out=deg[:, j:j + 1],
                    in_=wb[:, int(off[j]):int(off[j + 1])],
                    axis=mybir.AxisListType.X,
                )
        nc.vector.tensor_scalar_add(out=rec[:], in0=deg[:], scalar1=1.0)
        nc.vector.reciprocal(deg[:], rec[:])
        nc.scalar.sqrt(dinv[:], deg[:])
        for j in range(J):
            if off[j + 1] > off[j]:
                nc.vector.tensor_scalar_mul(
                    out=wt[:, int(off[j]):int(off[j + 1])],
                    in0=wb[:, int(off[j]):int(off[j + 1])],
                    scalar1=dinv[:, j:j + 1],
                )

        # ---- x' = dinv * x -> ag_in1 -> AllGather -> table1 ----
        nc.vector.tensor_tensor(
            out=zb[:].rearrange("p (j f) -> p j f", f=F),
            in0=xs[:].rearrange("p (j f) -> p j f", f=F),
            in1=dinv[:].unsqueeze(2).to_broadcast([P, J, F]),
            op=mybir.AluOpType.mult,
        )
        ag1_ap = ag1.ap().rearrange("(p j) f -> p (j f)", p=P)
        nc.sync.dma_start(out=ag1_ap, in_=zb[:])
        nc.gpsimd.collective_compute(
            "AllGather", mybir.AluOpType.bypass, replica_groups=groups,
            ins=[ag1.ap().opt()], outs=[table1.ap().opt()],
        )

        def aggregate(table):
            # HW indirect DMA honors one offset per partition per instruction
            # (the [P, 1] pattern), so gather one slot-column (128 rows) at a
            # time.  Empty (zero-degree) groups keep their memset slice.
            nc.vector.memset(agg[:], 0.0)
            for (j0, j1, o0, o1) in batches:
                S = o1 - o0
                g = gp.tile([P, maxS * F], f32, tag="g")
                for d in range(S):
                    nc.gpsimd.indirect_dma_start(
                        out=g[:, d * F:(d + 1) * F],
                        out_offset=None,
                        in_=table[:, :],
                        in_offset=bass.IndirectOffsetOnAxis(
                            ap=idxs[:, o0 + d:o0 + d + 1], axis=0),
                    )
                nc.vector.tensor_tensor(
                    out=g[:, :S * F].rearrange("p (s f) -> p s f", f=F),
                    in0=g[:, :S * F].rearrange("p (s f) -> p s f", f=F),
                    in1=wt[:, o0:o1].unsqueeze(2).to_broadcast([P, S, F]),
                    op=mybir.AluOpType.mult,
                )
                for j in range(j0, j1):
                    D = int(Dhat[j])
                    if D == 0:
                        continue
                    rel = int(off[j]) - o0
                    mj = g[:, rel * F:(rel + D) * F].rearrange(
                        "p (d f) -> p f d", f=F)
                    nc.vector.reduce_sum(
                        out=agg[:, j * F:(j + 1) * F],
                        in_=mj,
                        axis=mybir.AxisListType.X,
                    )

        def transform(Wt, bt, scale_dinv):
            for w0 in range(0, J, WAVE):
                w1 = min(w0 + WAVE, J)
                nW = w1 - w0
                # matmul input = agg + dinv * zb   (self-loop contribution:
                # zb holds this layer's dinv-prescaled input rows)
                tsf = ep.tile([P, WAVE * F], f32, tag="sf")
                nc.vector.tensor_tensor(
                    out=tsf[:, :nW * F].rearrange("p (j f) -> p j f", f=F),
                    in0=zb[:, w0 * F:w1 * F].rearrange("p (j f) -> p j f", f=F),
                    in1=dinv[:, w0:w1].unsqueeze(2).to_broadcast([P, nW, F]),
                    op=mybir.AluOpType.mult,
                )
                tsum = ep.tile([P, WAVE * F], f32, tag="ts")
                nc.vector.tensor_tensor(
                    out=tsum[:, :nW * F],
                    in0=tsf[:, :nW * F],
                    in1=agg[:, w0 * F:w1 * F],
                    op=mybir.AluOpType.add,
                )
                aggT = aT.tile([F, WAVE * P], f32, tag="aT")
                nhalf = math.ceil(nW / 4)
                for h in range(nhalf):
                    lo = w0 + h * 4
                    hi = min(lo + 4, w1)
                    psT = pT.tile([F, 4 * P], f32, tag="pT")
                    for i, j in enumerate(range(lo, hi)):
                        jj = j - w0
                        nc.tensor.transpose(
                            out=psT[:, i * P:(i + 1) * P],
                            in_=tsum[:, jj * F:(jj + 1) * F],
                            identity=ident[:],
                        )
                    nn = hi - lo
                    nc.vector.tensor_copy(
                        out=aggT[:, (h * 4) * P:(h * 4 + nn) * P],
                        in_=psT[:, :nn * P],
                    )
                psZ = pZ.tile([P, WAVE * F], f32, tag="pZ")
                for i, j in enumerate(range(w0, w1)):
                    nc.tensor.matmul(
                        out=psZ[:, i * F:(i + 1) * F],
                        lhsT=aggT[:, i * P:(i + 1) * P],
                        rhs=Wt[:],
                        start=True, stop=True,
                    )
                e1 = ep.tile([P, WAVE * F], f32, tag="e1")
                nc.vector.tensor_tensor(
                    out=e1[:, :nW * F].rearrange("p (j f) -> p j f", f=F),
                    in0=psZ[:, :nW * F].rearrange("p (j f) -> p j f", f=F),
                    in1=bt[:].unsqueeze(1).to_broadcast([P, nW, F]),
                    op=mybir.AluOpType.add,
                )
                if scale_dinv:
                    e2 = ep.tile([P, WAVE * F], f32, tag="e2")
                    nc.vector.tensor_tensor(
                        out=e2[:, :nW * F].rearrange("p (j f) -> p j f", f=F),
                        in0=e1[:, :nW * F].rearrange("p (j f) -> p j f", f=F),
                        in1=dinv[:, w0:w1].unsqueeze(2).to_broadcast([P, nW, F]),
                        op=mybir.AluOpType.mult,
                    )
                    src = e2
                else:
                    src = e1
                nc.scalar.activation(
                    out=zb[:, w0 * F:w1 * F],
                    in_=src[:, :nW * F],
                    func=mybir.ActivationFunctionType.Relu,
                )

        # ---- layer 1 ----
        with nc.named_scope("agg1"):
            aggregate(table1)
        with nc.named_scope("xform1"):
            transform(W1t, b1t, scale_dinv=True)
        with nc.named_scope("allgather2"):
            ag2_ap = ag2.ap().rearrange("(p j) f -> p (j f)", p=P)
            nc.sync.dma_start(out=ag2_ap, in_=zb[:])
            nc.gpsimd.collective_compute(
                "AllGather", mybir.AluOpType.bypass, replica_groups=groups,
                ins=[ag2.ap().opt()], outs=[table2.ap().opt()],
            )

        # ---- layer 2 ----
        with nc.named_scope("agg2"):
            aggregate(table2)
        with nc.named_scope("xform2"):
            transform(W2t, b2t, scale_dinv=False)
        nc.sync.dma_start(out=out_t[:, :], in_=zb[:])

    nc.compile()
    return nc


# ---------------------------------------------------------------------------
# Entry point
# ---------------------------------------------------------------------------

def _make_in_maps(plan, node_feats, W1, b1, W2, b2):
    x_slices = _shard_x(node_feats, plan)
    W1 = np.ascontiguousarray(np.asarray(W1, np.float32))
    W2 = np.ascontiguousarray(np.asarray(W2, np.float32))
    b1t = np.ascontiguousarray(np.broadcast_to(
        np.asarray(b1, np.float32)[None, :], (P, F)))
    b2t = np.ascontiguousarray(np.broadcast_to(
        np.asarray(b2, np.float32)[None, :], (P, F)))
    in_maps = []
    for k in range(C):
        in_maps.append({
            "x_slice": np.ascontiguousarray(x_slices[k]),
            "w_pad": np.ascontiguousarray(plan["w_pad"][k]),
            "idx": np.ascontiguousarray(plan["idx"][k]),
            "W1": W1, "W2": W2, "b1": b1t, "b2": b2t,
        })
    return in_maps


def _unshard(plan, outs):
    J, N = plan["J"], plan["N"]
    full = np.concatenate(
        [o.reshape(P, J, F).reshape(P * J, F) for o in outs], axis=0)
    return np.ascontiguousarray(full[plan["t_of"][:N]])


LAST_RESULT = None  # BassKernelResults of the most recent kernel() call


def kernel(node_feats, edge_index, edge_feats, W1, b1, W2, b2):
    global LAST_RESULT
    from concourse.bass_utils import run_bass_kernel_spmd

    plan = _plan(node_feats.shape[0], edge_index, edge_feats)
    nc = _build(plan)
    in_maps = _make_in_maps(plan, node_feats, W1, b1, W2, b2)
    res = run_bass_kernel_spmd(nc, in_maps, core_ids=list(range(C)))
    LAST_RESULT = res
    return _unshard(plan, [res.results[k]["out"] for k in range(C)])

